# revision 9
# baseline (speedup 1.0000x reference)
"""Trainium2 Bass kernel for a 2-layer GCN (EnhancedGNN).

Computation (eval mode):
    src,dst,norm = gcn_norm(edge_index)            # sym deg^-1/2 with self loops
    h  = relu(gcn_layer(x, W1, b1))
    h  = gcn_layer(h, W2, b2)
    out = sigmoid(h @ Wl + bl)

Key identity: the per-edge norm dinv[src]*dinv[dst] factors into per-node
row scales, so  layer(X) = dinv * segsum(hs[src] -> dst) + b  with
hs = dinv * (X @ W) and the self loop as an ordinary edge.

Distribution: nodes sharded over 8 cores (6250 real + 22 fake zero rows
-> 6272 slots/core).  Edges live on the dst-owner core, sorted by dst.
Each 128-node output group is segment-summed on TensorE: gathered
message chunks [128 slots, 128 feat] (bf16) are multiplied by constant
0/1 selection matrices and accumulated in PSUM.  Chunks are packed as
uniform-depth rectangles (d slots for 128//d consecutive nodes at a
base offset); a node's messages may span several chunks (PSUM
accumulates), which keeps padding small.  The chunk layout is built
from the elementwise-max (sorted) degree profile across cores so all 8
cores run one program.  Messages are fetched with batched dma_gather
(int16 indices -> two source banks of 25088 rows; a node's edges are
processed in two passes, one per bank).

The gather source (hs for all nodes, bf16) is assembled by each core
DMA-writing its own shard directly into a shared DRAM table at a
cc_rank-dependent offset; a tiny AllGather between strict barriers acts
as the cross-core fence (no bulk collective).  The inter-layer matmul
(hs2 = H1s @ W2) and the output projection/sigmoid are fused into the
per-group message-pass epilogues so they overlap the remaining gathers.
"""

import os
import sys

sys.path.insert(0, "/opt/trn_rl_repo")

import numpy as np

import concourse.bacc as bacc
import concourse.bass as bass
import concourse.tile as tile
from concourse import mybir
from concourse.ap import AP
from concourse.bass_utils import run_bass_kernel_spmd

# ---------------------------------------------------------------- constants
N_REAL = 50000
E_EDGES = 800000
D = 128                      # feature dim
NC = 8                       # cores
SHARD_REAL = N_REAL // NC    # 6250
G = 49                       # node groups of 128 per core
SHARD = G * 128              # 6272 slots per core (incl 22 fakes)
NP = NC * SHARD              # 50176 padded node rows
HALF = NP // 2               # 25088 = bank size (< 32768 for int16 idx)
GCHUNK = 48                  # chunks (of 128 slots) per dma_gather call
NQ = 4                       # SWDGE queues to round-robin gathers over

F32 = mybir.dt.float32
BF16 = mybir.dt.bfloat16
I16 = mybir.dt.int16


# ===================================================================== host
def _pack_profile(prof):
    """Greedy rectangle cover of a [128] need profile.

    Returns list of (d, base, take): d slots for each of `take` consecutive
    nodes starting at `base`.  A node may be covered by several chunks
    (PSUM accumulates).  Usable slots per chunk = (128//d)*d.
    """
    r = prof.astype(np.int64).copy()
    chunks = []
    while r.max() > 0:
        best = None
        for d in range(1, 129):
            cap = 128 // d
            if cap * d < 64 and d != 128:
                continue  # skip very wasteful odd sizes unless large d
            useful = np.minimum(r, d).astype(np.float64)
            if cap >= 128:
                u = useful.sum()
                b = 0
            else:
                c = np.convolve(useful, np.ones(cap), "valid")
                b = int(np.argmax(c))
                u = c[b]
            if best is None or u > best[0] + 1e-9:
                best = (u, d, b)
        _, d, b = best
        cap = 128 // d
        take = min(cap, 128 - b)
        chunks.append((d, b, take))
        r[b:b + take] = np.maximum(r[b:b + take] - d, 0)
    return chunks


def _host_prep(x, edge_index):
    """Build per-core sharded inputs + the uniform static schedule."""
    src = np.asarray(edge_index[0], dtype=np.int64)
    dst = np.asarray(edge_index[1], dtype=np.int64)

    deg = np.bincount(dst, minlength=N_REAL).astype(np.int64) + 1  # + self loop

    order = np.argsort(dst, kind="stable")
    s_src = src[order]
    s_dst = dst[order]
    starts = np.searchsorted(s_dst, np.arange(N_REAL), side="left")
    ends = np.searchsorted(s_dst, np.arange(N_REAL), side="right")

    src_bank = (s_src >= (SHARD_REAL * 4)).astype(np.int8)
    own_bank = (np.arange(N_REAL) >= (SHARD_REAL * 4)).astype(np.int64)

    n_in = np.zeros((N_REAL, 2), dtype=np.int64)
    np.add.at(n_in, (s_dst, src_bank.astype(np.int64)), 1)
    n_in[np.arange(N_REAL), own_bank] += 1  # self loop

    # ---- per-core permutation pi: sort slots by total need desc; fakes last
    pis = []
    needs = []          # per core [SHARD, 2] in pi order
    rows_of_real = np.full(N_REAL, -1, dtype=np.int64)
    for c in range(NC):
        lo = c * SHARD_REAL
        need = np.zeros((SHARD, 2), dtype=np.int64)
        need[:SHARD_REAL] = n_in[lo:lo + SHARD_REAL]
        tot = need[:, 0] + need[:, 1]
        key = -(tot * 512 + need[:, 0])
        pi = np.argsort(key, kind="stable")
        local = np.where(pi < SHARD_REAL, pi, -1)
        pis.append(local)
        needs.append(need[pi])
        mask = local >= 0
        rows_of_real[lo + local[mask]] = c * SHARD + np.nonzero(mask)[0]
    assert (rows_of_real >= 0).all()

    # all-zero pad rows, spread over many HBM addresses so pad reads don't
    # hotspot one DRAM channel
    fake_lists = [[], []]
    for c in range(NC):
        fslots = np.nonzero(pis[c] < 0)[0]
        fake_lists[c // 4].extend(c * SHARD + fslots)
    fake_lists = [np.array(f, dtype=np.int64) for f in fake_lists]
    assert all(len(f) > 0 for f in fake_lists)
    assert fake_lists[0].max() < HALF <= fake_lists[1].min()

    # ---- uniform max profile across cores, then chunk layouts
    max_need = np.maximum.reduce(needs)            # [SHARD, 2]

    layouts = {}
    for g in range(G):
        for p in range(2):
            prof = max_need[g * 128:(g + 1) * 128, p]
            layouts[(g, p)] = _pack_profile(prof)
    sched = []
    off = 0
    for p in range(2):
        for g in range(G):
            chunks = layouts[(g, p)]
            sched.append((g, p, chunks, off))
            off += 128 * len(chunks)
    tot_slots = off

    # ---- per-core gather indices
    idx_maps = []
    xT_maps = []
    deg_maps = []
    for c in range(NC):
        lo = c * SHARD_REAL
        # per-node src rows (bank-split), consumed in order across chunks
        node_rows = [[None, None] for _ in range(128)]  # per group scratch
        idx_flat = np.empty(tot_slots, dtype=np.int16)
        rr = 0
        # per (g, p) cursor into each node's row list
        for (g, p, chunks, o) in sched:
            fl = fake_lists[p] - p * HALF
            # build per-node row lists for this (g, p)
            rows_of = []
            cur = []
            for t in range(128):
                slot = g * 128 + t
                lreal = pis[c][slot]
                if lreal < 0:
                    rows_of.append(np.empty(0, dtype=np.int64))
                else:
                    v = lo + lreal
                    e0, e1 = starts[v], ends[v]
                    bsel = src_bank[e0:e1] == p
                    rows = rows_of_real[s_src[e0:e1][bsel]]
                    if own_bank[v] == p:
                        rows = np.concatenate([rows, [rows_of_real[v]]])
                    rows_of.append(rows - p * HALF)
                cur.append(0)
            for ci, (d, base, take) in enumerate(chunks):
                blk = fl[(rr + np.arange(128)) % len(fl)].copy()
                rr += 128
                usable_nodes = min(take, 128 // d)
                for t in range(usable_nodes):
                    node = base + t
                    rows = rows_of[node]
                    k = min(d, len(rows) - cur[node])
                    if k > 0:
                        blk[t * d:t * d + k] = rows[cur[node]:cur[node] + k]
                        cur[node] += k
                idx_flat[o + ci * 128:o + (ci + 1) * 128] = blk.astype(np.int16)
            for t in range(128):
                assert cur[t] == len(rows_of[t]), (c, g, p, t)
        wrapped = idx_flat.reshape(-1, 16).T.copy()
        idx_maps.append(np.tile(wrapped, (8, 1)))        # [128, tot/16]

        xT = np.zeros((D, SHARD), dtype=np.float32)
        mask = pis[c] >= 0
        xT[:, mask] = np.asarray(x)[lo + pis[c][mask]].T
        xT_maps.append(np.ascontiguousarray(xT))

        dg = np.ones(SHARD, dtype=np.float32)
        dg[mask] = deg[lo + pis[c][mask]].astype(np.float32)
        deg_maps.append(np.ascontiguousarray(dg.reshape(G, 128).T))

    # ---- selection matrices, one per distinct d.  Chunk at psum base b
    # uses slice [:, 127-b : 255-b]; ones sit at [s, 127 + s//d], s < m*d.
    d_set = sorted({d for chunks in layouts.values() for (d, _, _) in chunks})
    w_ext = {}
    for d in d_set:
        m = 128 // d
        w = np.zeros((128, 255), dtype=np.float32)
        s = np.arange(m * d)
        w[s, 127 + s // d] = 1.0
        w_ext[d] = w

    return dict(
        sched=sched, tot_slots=tot_slots, d_set=d_set, w_ext=w_ext,
        idx_maps=idx_maps, xT_maps=xT_maps, deg_maps=deg_maps,
        pis=pis, rows_of_real=rows_of_real, deg=deg,
    )


# ==================================================================== device
def _build_nc(prep, has_b1, has_b2):
    sched = prep["sched"]
    d_set = prep["d_set"]
    tot_slots = prep["tot_slots"]

    nc = bacc.Bacc("TRN2", target_bir_lowering=False, num_devices=NC,
                   num_swdge_queues=NQ)
    core_ids = list(range(NC))

    # ---- I/O
    xT_in = nc.declare_dram_parameter("xT", [D, SHARD], F32, isOutput=False)
    degg_in = nc.declare_dram_parameter("deg_g", [128, G], F32, isOutput=False)
    idx_in = nc.declare_dram_parameter(
        "idx_all", [128, tot_slots // 16], I16, isOutput=False)
    w1_in = nc.declare_dram_parameter("W1", [D, D], F32, isOutput=False)
    w2_in = nc.declare_dram_parameter("W2", [D, D], F32, isOutput=False)
    wlb_in = nc.declare_dram_parameter("Wl_bcast", [128, D], F32, isOutput=False)
    blr_in = nc.declare_dram_parameter("bl_rep", [128, 1], F32, isOutput=False)
    b1b_in = nc.declare_dram_parameter("b1_bcast", [128, D], F32, isOutput=False)
    b2b_in = nc.declare_dram_parameter("b2_bcast", [128, D], F32, isOutput=False)
    wexts_in = {
        d: nc.declare_dram_parameter(
            f"w_ext_{d}", [128, 255], BF16, isOutput=False)
        for d in d_set
    }
    ident_in = nc.declare_dram_parameter("ident", [128, 128], F32, isOutput=False)
    out_ext = nc.declare_dram_parameter("out", [SHARD, 1], F32, isOutput=True)

    # ---- internal DRAM (gather sources in bf16)
    hs1_shard = nc.dram_tensor("hs1_shard", [SHARD, D], BF16)
    hs2_shard = nc.dram_tensor("hs2_shard", [SHARD, D], BF16)
    hs1_ag = nc.dram_tensor("hs1_ag", [NP, D], BF16, addr_space="Shared")
    hs2_ag = nc.dram_tensor("hs2_ag", [NP, D], BF16, addr_space="Shared")

    from contextlib import ExitStack
    with tile.TileContext(nc) as tc, ExitStack() as es:
        cpool = es.enter_context(tc.tile_pool(name="const", bufs=1))
        gpool = es.enter_context(tc.tile_pool(name="gather", bufs=4))
        spool = es.enter_context(tc.tile_pool(name="stage", bufs=4))
        ppool = es.enter_context(tc.tile_pool(name="psum", bufs=4, space="PSUM"))
        ppool2 = es.enter_context(tc.tile_pool(name="psum2", bufs=2, space="PSUM"))

        # ---------------- phase-B-critical constants
        xT_t = cpool.tile([D, SHARD], F32, tag="xT")
        nc.sync.dma_start(out=xT_t[:], in_=xT_in[:])
        w1_t = cpool.tile([D, D], F32, tag="w1")
        nc.sync.dma_start(out=w1_t[:], in_=w1_in[:])
        degg_t = cpool.tile([128, G], F32, tag="degg")
        nc.sync.dma_start(out=degg_t[:], in_=degg_in[:])

        tc.strict_bb_all_engine_barrier()

        sdeg_t = cpool.tile([128, G], F32, tag="sdeg")
        nc.scalar.sqrt(sdeg_t[:], degg_t[:])
        dinv_t = cpool.tile([128, G], F32, tag="dinv")
        nc.vector.reciprocal(dinv_t[:], sdeg_t[:])

        # ---------------- phase B: hs1 = bf16(dinv * (x @ W1)) -> shared tab
        for g in range(G):
            ps = ppool2.tile([128, D], F32, space="PSUM", tag="mmps")
            nc.tensor.matmul(ps[:], lhsT=xT_t[:, g * 128:(g + 1) * 128],
                             rhs=w1_t[:], start=True, stop=True)
            st = spool.tile([128, D], BF16, tag="bstage")
            nc.scalar.activation(st[:], ps[:], mybir.ActivationFunctionType.Copy,
                                 bias=0.0, scale=dinv_t[:, g:g + 1])
            nc.sync.dma_start(out=hs1_shard[g * 128:(g + 1) * 128, :], in_=st[:])

        # ---------------- remaining constants (overlap with phase B)
        w2_t = cpool.tile([D, D], F32, tag="w2")
        nc.sync.dma_start(out=w2_t[:], in_=w2_in[:])
        wlb_t = cpool.tile([128, D], F32, tag="wlb")
        nc.sync.dma_start(out=wlb_t[:], in_=wlb_in[:])
        blr_t = cpool.tile([128, 1], F32, tag="blr")
        nc.sync.dma_start(out=blr_t[:], in_=blr_in[:])
        b1b_t = cpool.tile([128, D], F32, tag="b1b")
        nc.sync.dma_start(out=b1b_t[:], in_=b1b_in[:])
        b2b_t = cpool.tile([128, D], F32, tag="b2b")
        nc.sync.dma_start(out=b2b_t[:], in_=b2b_in[:])
        idx_t = cpool.tile([128, tot_slots // 16], I16, tag="idx")
        nc.sync.dma_start(out=idx_t[:], in_=idx_in[:])
        wext_t = {}
        for d in d_set:
            t = cpool.tile([128, 255], BF16, tag=f"wext{d}")
            nc.sync.dma_start(out=t[:], in_=wexts_in[d][:])
            wext_t[d] = t
        ident_t = cpool.tile([128, 128], F32, tag="ident")
        nc.sync.dma_start(out=ident_t[:], in_=ident_in[:])

        h1s_all = cpool.tile([128, G * D], F32, tag="h1s")
        h2_parked = cpool.tile([128, G * D], F32, tag="h2p")

        nc.gpsimd.collective_compute(
            "AllGather", mybir.AluOpType.bypass,
            replica_groups=[core_ids],
            ins=[hs1_shard[:]], outs=[hs1_ag[:]],
        )

        qctr = [0]
        A = mybir.ActivationFunctionType

        # ---------------- message passing (shared by both layers)
        def message_pass(src_ag, parked, layer):
            banks = [src_ag[0:HALF, :], src_ag[HALF:NP, :]]

            def epilogue(g, ps):
                dv = dinv_t[:, g:g + 1]
                dst = parked[:, g * D:(g + 1) * D]
                t0 = spool.tile([128, D], F32, tag="ep0")
                nc.vector.tensor_add(t0[:], ps[:], dst)
                if layer == 1:
                    # H1s = dinv * relu(dinv*seg + b1);  (b1 known zero or
                    # handled via b1b when has_b1)
                    if has_b1:
                        t1 = spool.tile([128, D], F32, tag="ep1")
                        nc.scalar.activation(t1[:], t0[:], A.Copy,
                                             bias=0.0, scale=dv)
                        t2 = spool.tile([128, D], F32, tag="ep2")
                        nc.vector.tensor_add(t2[:], t1[:], b1b_t[:])
                        t3 = spool.tile([128, D], F32, tag="ep3")
                        nc.scalar.activation(t3[:], t2[:], A.Relu)
                        h1s = spool.tile([128, D], F32, tag="ep4")
                        nc.scalar.activation(h1s[:], t3[:], A.Copy,
                                             bias=0.0, scale=dv)
                    else:
                        t1 = spool.tile([128, D], F32, tag="ep1")
                        nc.scalar.activation(t1[:], t0[:], A.Relu,
                                             bias=0.0, scale=dv)
                        h1s = spool.tile([128, D], F32, tag="ep4")
                        nc.scalar.activation(h1s[:], t1[:], A.Copy,
                                             bias=0.0, scale=dv)
                    nc.vector.tensor_copy(h1s_all[:, g * D:(g + 1) * D],
                                          h1s[:])
                    # fused phase D: hs2 = bf16(H1s @ W2)
                    pt = ppool2.tile([128, D], F32, space="PSUM", tag="tps")
                    nc.tensor.transpose(pt[:], h1s[:], ident_t[:])
                    tt = spool.tile([128, D], F32, tag="ttile")
                    nc.vector.tensor_copy(tt[:], pt[:])
                    ps2 = ppool2.tile([128, D], F32, space="PSUM", tag="mmps")
                    nc.tensor.matmul(ps2[:], lhsT=tt[:], rhs=w2_t[:],
                                     start=True, stop=True)
                    st = spool.tile([128, D], BF16, tag="bstage")
                    nc.vector.tensor_copy(st[:], ps2[:])
                    nc.sync.dma_start(out=hs2_shard[g * 128:(g + 1) * 128, :],
                                      in_=st[:])
                else:
                    # fused phase F: out = sigmoid(dinv*(seg@Wl) (+b2@Wl) +bl)
                    if has_b2:
                        tb = spool.tile([128, D], F32, tag="ep1")
                        nc.scalar.activation(tb[:], t0[:], A.Copy,
                                             bias=0.0, scale=dv)
                        t2 = spool.tile([128, D], F32, tag="ep2")
                        nc.vector.tensor_add(t2[:], tb[:], b2b_t[:])
                        mt = spool.tile([128, D], F32, tag="fmul")
                        nc.vector.tensor_tensor(out=mt[:], in0=t2[:],
                                                in1=wlb_t[:],
                                                op=mybir.AluOpType.mult)
                        rt = spool.tile([128, 1], F32, tag="fred")
                        nc.vector.tensor_reduce(rt[:], mt[:],
                                                axis=mybir.AxisListType.X,
                                                op=mybir.AluOpType.add)
                        ot = spool.tile([128, 1], F32, tag="fout")
                        nc.scalar.activation(ot[:], rt[:], A.Sigmoid,
                                             bias=blr_t[:], scale=1.0)
                    else:
                        mt = spool.tile([128, D], F32, tag="fmul")
                        nc.vector.tensor_tensor(out=mt[:], in0=t0[:],
                                                in1=wlb_t[:],
                                                op=mybir.AluOpType.mult)
                        rt = spool.tile([128, 1], F32, tag="fred")
                        nc.vector.tensor_reduce(rt[:], mt[:],
                                                axis=mybir.AxisListType.X,
                                                op=mybir.AluOpType.add)
                        ot = spool.tile([128, 1], F32, tag="fout")
                        nc.scalar.activation(ot[:], rt[:], A.Sigmoid,
                                             bias=blr_t[:], scale=dv)
                    nc.sync.dma_start(out=out_ext[g * 128:(g + 1) * 128, :],
                                      in_=ot[:])

            for p in range(2):
                flat = []           # (g, d, base, last_of_group)
                base_off = None
                for (gg, pp, chunks, o) in sched:
                    if pp != p:
                        continue
                    if base_off is None:
                        base_off = o
                    for ci, (d, base, take) in enumerate(chunks):
                        flat.append((gg, d, base, ci + 1 == len(chunks)))
                cur_ps = {}
                for w0 in range(0, len(flat), GCHUNK):
                    wchunks = flat[w0:w0 + GCHUNK]
                    ncnk = len(wchunks)
                    gt = gpool.tile([128, GCHUNK * D], BF16, tag="gmsg")
                    n_idx = ncnk * 128
                    q = qctr[0] % NQ
                    qctr[0] += 1
                    o0 = base_off + w0 * 128
                    nc.gpsimd.dma_gather(
                        gt[:, :ncnk * D].rearrange("p (c f) -> p c f", f=D),
                        banks[p],
                        idx_t[:, o0 // 16:(o0 + ncnk * 128) // 16],
                        n_idx, n_idx, D, queue_num=q, single_packet=False,
                    )
                    for ci, (g, d, base, last) in enumerate(wchunks):
                        if g not in cur_ps:
                            segps = ppool.tile([128, D], F32, space="PSUM",
                                               tag="segps",
                                               name=f"segps_{layer}_{p}_{g}")
                            cur_ps[g] = (segps, True)
                        ps, first = cur_ps[g]
                        nc.tensor.matmul(
                            ps[:],
                            lhsT=wext_t[d][:, 127 - base:255 - base],
                            rhs=gt[:, ci * D:(ci + 1) * D],
                            start=first, stop=last,
                        )
                        cur_ps[g] = (ps, False)
                        if last:
                            if p == 0:
                                nc.scalar.activation(
                                    parked[:, g * D:(g + 1) * D], ps[:],
                                    A.Copy)
                            else:
                                epilogue(g, ps)
                            del cur_ps[g]

        # layer 1 (epilogue writes hs2_shard slices)
        message_pass(hs1_ag, h1s_all, layer=1)

        nc.gpsimd.collective_compute(
            "AllGather", mybir.AluOpType.bypass,
            replica_groups=[core_ids],
            ins=[hs2_shard[:]], outs=[hs2_ag[:]],
        )

        # layer 2 (epilogue writes final outputs)
        message_pass(hs2_ag, h2_parked, layer=2)

    nc.compile()
    return nc


# ==================================================================== entry
_CACHE = {}


def kernel(x, edge_index, W1, b1, W2, b2, Wl, bl):
    import ml_dtypes  # noqa: F401  (registers bfloat16 with numpy)

    x = np.asarray(x, dtype=np.float32)
    edge_index = np.asarray(edge_index)
    W1 = np.asarray(W1, dtype=np.float32)
    W2 = np.asarray(W2, dtype=np.float32)
    Wl = np.asarray(Wl, dtype=np.float32)
    b1 = np.asarray(b1, dtype=np.float32)
    b2 = np.asarray(b2, dtype=np.float32)
    bl = np.asarray(bl, dtype=np.float32)

    prep = _host_prep(x, edge_index)
    has_b1 = bool(np.any(b1))
    has_b2 = bool(np.any(b2))

    nc = _build_nc(prep, has_b1, has_b2)

    wl_bcast = np.tile(Wl.reshape(1, D), (128, 1)).astype(np.float32)
    bl_rep = np.full((128, 1), float(bl.reshape(-1)[0]), dtype=np.float32)
    b1_bcast = np.tile(b1.reshape(1, D), (128, 1)).astype(np.float32)
    b2_bcast = np.tile(b2.reshape(1, D), (128, 1)).astype(np.float32)

    import ml_dtypes as mld
    in_maps = []
    for c in range(NC):
        m = {
            "xT": prep["xT_maps"][c],
            "deg_g": prep["deg_maps"][c],
            "idx_all": prep["idx_maps"][c],
            "W1": W1, "W2": W2,
            "Wl_bcast": wl_bcast, "bl_rep": bl_rep,
            "b1_bcast": b1_bcast, "b2_bcast": b2_bcast,
        }
        for d, w in prep["w_ext"].items():
            m[f"w_ext_{d}"] = np.asarray(w, dtype=mld.bfloat16)
        m["ident"] = np.eye(128, dtype=np.float32)
        in_maps.append(m)

    trace = bool(os.environ.get("GNN_TRACE"))
    kw = {}
    if trace:
        kw = dict(trace=True, tmpdir=os.environ.get("GNN_TRACE_DIR") or None)
    res = run_bass_kernel_spmd(nc, in_maps, list(range(NC)), **kw)
    _CACHE["last_result"] = res

    out = np.empty((N_REAL, 1), dtype=np.float32)
    for c in range(NC):
        o = res.results[c]["out"]          # [SHARD, 1], pi order
        pi = prep["pis"][c]
        mask = pi >= 0
        out[c * SHARD_REAL + pi[mask], 0] = o[mask, 0]
    return out


if __name__ == "__main__":
    rng = np.random.default_rng(0)
    x = rng.standard_normal((N_REAL, D), dtype=np.float32)
    ei = rng.integers(0, N_REAL, size=(2, E_EDGES), dtype=np.int64)
    W1 = rng.standard_normal((D, D), dtype=np.float32) / np.sqrt(D)
    W2 = rng.standard_normal((D, D), dtype=np.float32) / np.sqrt(D)
    Wl = rng.standard_normal((D, 1), dtype=np.float32) / np.sqrt(D)
    z = np.zeros(D, dtype=np.float32)
    out = kernel(x=x, edge_index=ei, W1=W1, b1=z, W2=W2, b2=z,
                 Wl=Wl, bl=np.zeros(1, dtype=np.float32))
    print(out.shape, out[:5, 0])


# revision 21
# speedup vs baseline: 1.0277x; 1.0277x over previous
"""Trainium2 Bass kernel for a 2-layer GCN (EnhancedGNN).

Computation (eval mode):
    src,dst,norm = gcn_norm(edge_index)            # sym deg^-1/2 with self loops
    h  = relu(gcn_layer(x, W1, b1))
    h  = gcn_layer(h, W2, b2)
    out = sigmoid(h @ Wl + bl)

Key identity: the per-edge norm dinv[src]*dinv[dst] factors into per-node
row scales, so  layer(X) = dinv * segsum(hs[src] -> dst) + b  with
hs = dinv * (X @ W) and the self loop as an ordinary edge.

Distribution: nodes sharded over 8 cores (6250 real + 22 fake zero rows
-> 6272 slots/core).  Edges live on the dst-owner core, sorted by dst.
Each 128-node output group is segment-summed on TensorE: gathered
message chunks [128 slots, 128 feat] (bf16) are multiplied by constant
0/1 selection matrices and accumulated in PSUM.  Chunks are packed as
uniform-depth rectangles (d slots for 128//d consecutive nodes at a
base offset); a node's messages may span several chunks (PSUM
accumulates), which keeps padding small.  The chunk layout is built
from the elementwise-max (sorted) degree profile across cores so all 8
cores run one program.  Messages are fetched with batched dma_gather
(int16 indices -> two source banks of 25088 rows; a node's edges are
processed in two passes, one per bank).

The gather source (hs for all nodes, bf16) is assembled by each core
DMA-writing its own shard directly into a shared DRAM table at a
cc_rank-dependent offset; a tiny AllGather between strict barriers acts
as the cross-core fence (no bulk collective).  The inter-layer matmul
(hs2 = H1s @ W2) and the output projection/sigmoid are fused into the
per-group message-pass epilogues so they overlap the remaining gathers.
"""

import os
import sys

sys.path.insert(0, "/opt/trn_rl_repo")

import numpy as np

import concourse.bacc as bacc
import concourse.bass as bass
import concourse.tile as tile
from concourse import mybir
from concourse.ap import AP
from concourse.bass_utils import run_bass_kernel_spmd

# ---------------------------------------------------------------- constants
N_REAL = 50000
E_EDGES = 800000
D = 128                      # feature dim
NC = 8                       # cores
SHARD_REAL = N_REAL // NC    # 6250
G = 49                       # node groups of 128 per core
SHARD = G * 128              # 6272 slots per core (incl 22 fakes)
NP = NC * SHARD              # 50176 padded node rows
HALF = NP // 2               # 25088 = bank size (< 32768 for int16 idx)
GCHUNK = 32                  # chunks (of 128 slots) per dma_gather call
NQ = 4                       # SWDGE queues to round-robin gathers over

F32 = mybir.dt.float32
BF16 = mybir.dt.bfloat16
I16 = mybir.dt.int16


# ===================================================================== host
def _pack_profile(prof):
    """Greedy rectangle cover of a [128] need profile.

    Returns list of (d, base, take): d slots for each of `take` consecutive
    nodes starting at `base`.  A node may be covered by several chunks
    (PSUM accumulates).  Usable slots per chunk = (128//d)*d.
    """
    r = prof.astype(np.int64).copy()
    chunks = []
    while r.max() > 0:
        best = None
        for d in range(1, 129):
            cap = 128 // d
            if cap * d < 64 and d != 128:
                continue  # skip very wasteful odd sizes unless large d
            useful = np.minimum(r, d).astype(np.float64)
            if cap >= 128:
                u = useful.sum()
                b = 0
            else:
                c = np.convolve(useful, np.ones(cap), "valid")
                b = int(np.argmax(c))
                u = c[b]
            if best is None or u > best[0] + 1e-9:
                best = (u, d, b)
        _, d, b = best
        cap = 128 // d
        take = min(cap, 128 - b)
        chunks.append((d, b, take))
        r[b:b + take] = np.maximum(r[b:b + take] - d, 0)
    return chunks


def _host_prep(x, edge_index):
    """Build per-core sharded inputs + the uniform static schedule."""
    src = np.asarray(edge_index[0], dtype=np.int64)
    dst = np.asarray(edge_index[1], dtype=np.int64)

    deg = np.bincount(dst, minlength=N_REAL).astype(np.int64) + 1  # + self loop

    order = np.argsort(dst, kind="stable")
    s_src = src[order]
    s_dst = dst[order]
    starts = np.searchsorted(s_dst, np.arange(N_REAL), side="left")
    ends = np.searchsorted(s_dst, np.arange(N_REAL), side="right")

    src_bank = (s_src >= (SHARD_REAL * 4)).astype(np.int8)
    own_bank = (np.arange(N_REAL) >= (SHARD_REAL * 4)).astype(np.int64)

    n_in = np.zeros((N_REAL, 2), dtype=np.int64)
    np.add.at(n_in, (s_dst, src_bank.astype(np.int64)), 1)
    n_in[np.arange(N_REAL), own_bank] += 1  # self loop

    # ---- per-core permutation pi: sort slots by total need desc; fakes last
    pis = []
    needs = []          # per core [SHARD, 2] in pi order
    rows_of_real = np.full(N_REAL, -1, dtype=np.int64)
    for c in range(NC):
        lo = c * SHARD_REAL
        need = np.zeros((SHARD, 2), dtype=np.int64)
        need[:SHARD_REAL] = n_in[lo:lo + SHARD_REAL]
        tot = need[:, 0] + need[:, 1]
        key = -(tot * 512 + need[:, 0])
        pi = np.argsort(key, kind="stable")
        local = np.where(pi < SHARD_REAL, pi, -1)
        pis.append(local)
        needs.append(need[pi])
        mask = local >= 0
        rows_of_real[lo + local[mask]] = c * SHARD + np.nonzero(mask)[0]
    assert (rows_of_real >= 0).all()

    # all-zero pad rows, spread over many HBM addresses so pad reads don't
    # hotspot one DRAM channel
    fake_lists = [[], []]
    for c in range(NC):
        fslots = np.nonzero(pis[c] < 0)[0]
        fake_lists[c // 4].extend(c * SHARD + fslots)
    fake_lists = [np.array(f, dtype=np.int64) for f in fake_lists]
    assert all(len(f) > 0 for f in fake_lists)
    assert fake_lists[0].max() < HALF <= fake_lists[1].min()

    # ---- uniform max profile across cores, then chunk layouts
    max_need = np.maximum.reduce(needs)            # [SHARD, 2]

    layouts = {}
    for g in range(G):
        for p in range(2):
            prof = max_need[g * 128:(g + 1) * 128, p]
            layouts[(g, p)] = _pack_profile(prof)
    sched = []
    off = 0
    for p in range(2):
        for g in range(G):
            chunks = layouts[(g, p)]
            sched.append((g, p, chunks, off))
            off += 128 * len(chunks)
    tot_slots = off

    # ---- per-core gather indices
    idx_maps = []
    xT_maps = []
    deg_maps = []
    for c in range(NC):
        lo = c * SHARD_REAL
        # per-node src rows (bank-split), consumed in order across chunks
        node_rows = [[None, None] for _ in range(128)]  # per group scratch
        idx_flat = np.empty(tot_slots, dtype=np.int16)
        rr = 0
        # per (g, p) cursor into each node's row list
        for (g, p, chunks, o) in sched:
            fl = fake_lists[p] - p * HALF
            # build per-node row lists for this (g, p)
            rows_of = []
            cur = []
            for t in range(128):
                slot = g * 128 + t
                lreal = pis[c][slot]
                if lreal < 0:
                    rows_of.append(np.empty(0, dtype=np.int64))
                else:
                    v = lo + lreal
                    e0, e1 = starts[v], ends[v]
                    bsel = src_bank[e0:e1] == p
                    rows = rows_of_real[s_src[e0:e1][bsel]]
                    if own_bank[v] == p:
                        rows = np.concatenate([rows, [rows_of_real[v]]])
                    rows_of.append(rows - p * HALF)
                cur.append(0)
            for ci, (d, base, take) in enumerate(chunks):
                blk = fl[(rr + np.arange(128)) % len(fl)].copy()
                rr += 128
                usable_nodes = min(take, 128 // d)
                for t in range(usable_nodes):
                    node = base + t
                    rows = rows_of[node]
                    k = min(d, len(rows) - cur[node])
                    if k > 0:
                        blk[t * d:t * d + k] = rows[cur[node]:cur[node] + k]
                        cur[node] += k
                idx_flat[o + ci * 128:o + (ci + 1) * 128] = blk.astype(np.int16)
            for t in range(128):
                assert cur[t] == len(rows_of[t]), (c, g, p, t)
        wrapped = idx_flat.reshape(-1, 16).T.copy()
        idx_maps.append(np.tile(wrapped, (8, 1)))        # [128, tot/16]

        import ml_dtypes as mld
        xT = np.zeros((D, SHARD), dtype=np.float32)
        mask = pis[c] >= 0
        xT[:, mask] = np.asarray(x)[lo + pis[c][mask]].T
        xT_maps.append(np.ascontiguousarray(xT).astype(mld.bfloat16))

        dg = np.ones(SHARD, dtype=np.float32)
        dg[mask] = deg[lo + pis[c][mask]].astype(np.float32)
        deg_maps.append(np.ascontiguousarray(dg.reshape(G, 128).T))

    # ---- selection matrices, one per distinct d.  Chunk at psum base b
    # uses slice [:, 127-b : 255-b]; ones sit at [s, 127 + s//d], s < m*d.
    d_set = sorted({d for chunks in layouts.values() for (d, _, _) in chunks})
    w_ext = {}
    for d in d_set:
        m = 128 // d
        w = np.zeros((128, 255), dtype=np.float32)
        s = np.arange(m * d)
        w[s, 127 + s // d] = 1.0
        w_ext[d] = w

    return dict(
        sched=sched, tot_slots=tot_slots, d_set=d_set, w_ext=w_ext,
        idx_maps=idx_maps, xT_maps=xT_maps, deg_maps=deg_maps,
        pis=pis, rows_of_real=rows_of_real, deg=deg,
    )


# ==================================================================== device
def _build_nc(prep, has_b1, has_b2):
    sched = prep["sched"]
    d_set = prep["d_set"]
    tot_slots = prep["tot_slots"]

    nc = bacc.Bacc("TRN2", target_bir_lowering=False, num_devices=NC,
                   num_swdge_queues=NQ)
    core_ids = list(range(NC))

    # ---- I/O
    xT_in = nc.declare_dram_parameter("xT", [D, SHARD], BF16, isOutput=False)
    degg_in = nc.declare_dram_parameter("deg_g", [128, G], F32, isOutput=False)
    idx_in = nc.declare_dram_parameter(
        "idx_all", [128, tot_slots // 16], I16, isOutput=False)
    w1_in = nc.declare_dram_parameter("W1", [D, D], F32, isOutput=False)
    w2_in = nc.declare_dram_parameter("W2", [D, D], F32, isOutput=False)
    wlb_in = nc.declare_dram_parameter("Wl_bcast", [128, D], F32, isOutput=False)
    blr_in = nc.declare_dram_parameter("bl_rep", [128, 1], F32, isOutput=False)
    b1b_in = nc.declare_dram_parameter("b1_bcast", [128, D], F32, isOutput=False)
    b2b_in = nc.declare_dram_parameter("b2_bcast", [128, D], F32, isOutput=False)
    wexts_in = {
        d: nc.declare_dram_parameter(
            f"w_ext_{d}", [128, 255], BF16, isOutput=False)
        for d in d_set
    }
    ident_in = nc.declare_dram_parameter("ident", [128, 128], F32, isOutput=False)
    out_ext = nc.declare_dram_parameter("out", [SHARD, 1], F32, isOutput=True)

    # ---- internal DRAM (gather sources in bf16)
    hs1_shard = nc.dram_tensor("hs1_shard", [SHARD, D], BF16)
    hs2_shard = nc.dram_tensor("hs2_shard", [SHARD, D], BF16)
    hs1_ag = nc.dram_tensor("hs1_ag", [NP, D], BF16, addr_space="Shared")
    hs2_ag = nc.dram_tensor("hs2_ag", [NP, D], BF16, addr_space="Shared")

    from contextlib import ExitStack
    with tile.TileContext(nc) as tc, ExitStack() as es:
        cpool = es.enter_context(tc.tile_pool(name="const", bufs=1))
        gpool = es.enter_context(tc.tile_pool(name="gather", bufs=4))
        spool = es.enter_context(tc.tile_pool(name="stage", bufs=4))
        ppool = es.enter_context(tc.tile_pool(name="psum", bufs=4, space="PSUM"))
        ppool2 = es.enter_context(tc.tile_pool(name="psum2", bufs=2, space="PSUM"))

        # ---------------- phase-B-critical constants
        xT_t = cpool.tile([D, SHARD], BF16, tag="xT")
        nc.sync.dma_start(out=xT_t[:], in_=xT_in[:])
        w1_t = cpool.tile([D, D], F32, tag="w1")
        nc.sync.dma_start(out=w1_t[:], in_=w1_in[:])
        w1b_t = cpool.tile([D, D], BF16, tag="w1b")
        nc.vector.tensor_copy(w1b_t[:], w1_t[:])
        degg_t = cpool.tile([128, G], F32, tag="degg")
        nc.sync.dma_start(out=degg_t[:], in_=degg_in[:])

        tc.strict_bb_all_engine_barrier()

        sdeg_t = cpool.tile([128, G], F32, tag="sdeg")
        nc.scalar.sqrt(sdeg_t[:], degg_t[:])
        dinv_t = cpool.tile([128, G], F32, tag="dinv")
        nc.vector.reciprocal(dinv_t[:], sdeg_t[:])

        # ---------------- phase B: hs1 = bf16(dinv * (x @ W1)) -> shard
        GSPLIT = 40            # AllGather split point (groups)
        for g in range(G):
            ps = ppool2.tile([128, D], F32, space="PSUM", tag="mmps")
            nc.tensor.matmul(ps[:], lhsT=xT_t[:, g * 128:(g + 1) * 128],
                             rhs=w1b_t[:], start=True, stop=True)
            st = spool.tile([128, D], BF16, tag="bstage")
            nc.scalar.activation(st[:], ps[:], mybir.ActivationFunctionType.Copy,
                                 bias=0.0, scale=dinv_t[:, g:g + 1])
            nc.sync.dma_start(out=hs1_shard[g * 128:(g + 1) * 128, :], in_=st[:])

        # ---------------- remaining constants (overlap with phase B)
        w2_t = cpool.tile([D, D], F32, tag="w2")
        nc.sync.dma_start(out=w2_t[:], in_=w2_in[:])
        wlb_t = cpool.tile([128, D], F32, tag="wlb")
        nc.sync.dma_start(out=wlb_t[:], in_=wlb_in[:])
        blr_t = cpool.tile([128, 1], F32, tag="blr")
        nc.sync.dma_start(out=blr_t[:], in_=blr_in[:])
        b1b_t = cpool.tile([128, D], F32, tag="b1b")
        nc.sync.dma_start(out=b1b_t[:], in_=b1b_in[:])
        b2b_t = cpool.tile([128, D], F32, tag="b2b")
        nc.sync.dma_start(out=b2b_t[:], in_=b2b_in[:])
        idx_t = cpool.tile([128, tot_slots // 16], I16, tag="idx")
        nc.sync.dma_start(out=idx_t[:], in_=idx_in[:])
        wext_t = {}
        for d in d_set:
            t = cpool.tile([128, 255], BF16, tag=f"wext{d}")
            nc.sync.dma_start(out=t[:], in_=wexts_in[d][:])
            wext_t[d] = t
        ident_t = cpool.tile([128, 128], F32, tag="ident")
        nc.sync.dma_start(out=ident_t[:], in_=ident_in[:])

        h1s_all = cpool.tile([128, G * D], F32, tag="h1s")
        h2_parked = cpool.tile([128, G * D], F32, tag="h2p")

        nc.gpsimd.collective_compute(
            "AllGather", mybir.AluOpType.bypass,
            replica_groups=[core_ids],
            ins=[hs1_shard[:]], outs=[hs1_ag[:]],
        )

        qctr = [0]
        A = mybir.ActivationFunctionType

        # ---------------- message passing (shared by both layers)
        def message_pass(src_ag, parked, layer):
            banks = [src_ag[0:HALF, :], src_ag[HALF:NP, :]]

            def epilogue(g, ps):
                dv = dinv_t[:, g:g + 1]
                dst = parked[:, g * D:(g + 1) * D]
                t0 = spool.tile([128, D], F32, tag="ep0")
                nc.vector.tensor_add(t0[:], ps[:], dst)
                if layer == 1:
                    # H1s = dinv * relu(dinv*seg + b1);  (b1 known zero or
                    # handled via b1b when has_b1)
                    if has_b1:
                        t1 = spool.tile([128, D], F32, tag="ep1")
                        nc.scalar.activation(t1[:], t0[:], A.Copy,
                                             bias=0.0, scale=dv)
                        t2 = spool.tile([128, D], F32, tag="ep2")
                        nc.vector.tensor_add(t2[:], t1[:], b1b_t[:])
                        t3 = spool.tile([128, D], F32, tag="ep3")
                        nc.scalar.activation(t3[:], t2[:], A.Relu)
                        h1s = spool.tile([128, D], F32, tag="ep4")
                        nc.scalar.activation(h1s[:], t3[:], A.Copy,
                                             bias=0.0, scale=dv)
                    else:
                        t1 = spool.tile([128, D], F32, tag="ep1")
                        nc.scalar.activation(t1[:], t0[:], A.Relu,
                                             bias=0.0, scale=dv)
                        h1s = spool.tile([128, D], F32, tag="ep4")
                        nc.scalar.activation(h1s[:], t1[:], A.Copy,
                                             bias=0.0, scale=dv)
                    nc.vector.tensor_copy(h1s_all[:, g * D:(g + 1) * D],
                                          h1s[:])
                    # fused phase D: hs2 = bf16(H1s @ W2)
                    pt = ppool2.tile([128, D], F32, space="PSUM", tag="tps")
                    nc.tensor.transpose(pt[:], h1s[:], ident_t[:])
                    tt = spool.tile([128, D], F32, tag="ttile")
                    nc.vector.tensor_copy(tt[:], pt[:])
                    ps2 = ppool2.tile([128, D], F32, space="PSUM", tag="mmps")
                    nc.tensor.matmul(ps2[:], lhsT=tt[:], rhs=w2_t[:],
                                     start=True, stop=True)
                    st = spool.tile([128, D], BF16, tag="bstage")
                    nc.vector.tensor_copy(st[:], ps2[:])
                    nc.sync.dma_start(out=hs2_shard[g * 128:(g + 1) * 128, :],
                                      in_=st[:])
                else:
                    # fused phase F: out = sigmoid(dinv*(seg@Wl) (+b2@Wl) +bl)
                    if has_b2:
                        tb = spool.tile([128, D], F32, tag="ep1")
                        nc.scalar.activation(tb[:], t0[:], A.Copy,
                                             bias=0.0, scale=dv)
                        t2 = spool.tile([128, D], F32, tag="ep2")
                        nc.vector.tensor_add(t2[:], tb[:], b2b_t[:])
                        mt = spool.tile([128, D], F32, tag="fmul")
                        nc.vector.tensor_tensor(out=mt[:], in0=t2[:],
                                                in1=wlb_t[:],
                                                op=mybir.AluOpType.mult)
                        rt = spool.tile([128, 1], F32, tag="fred")
                        nc.vector.tensor_reduce(rt[:], mt[:],
                                                axis=mybir.AxisListType.X,
                                                op=mybir.AluOpType.add)
                        ot = spool.tile([128, 1], F32, tag="fout")
                        nc.scalar.activation(ot[:], rt[:], A.Sigmoid,
                                             bias=blr_t[:], scale=1.0)
                    else:
                        mt = spool.tile([128, D], F32, tag="fmul")
                        nc.vector.tensor_tensor(out=mt[:], in0=t0[:],
                                                in1=wlb_t[:],
                                                op=mybir.AluOpType.mult)
                        rt = spool.tile([128, 1], F32, tag="fred")
                        nc.vector.tensor_reduce(rt[:], mt[:],
                                                axis=mybir.AxisListType.X,
                                                op=mybir.AluOpType.add)
                        ot = spool.tile([128, 1], F32, tag="fout")
                        nc.scalar.activation(ot[:], rt[:], A.Sigmoid,
                                             bias=blr_t[:], scale=dv)
                    nc.sync.dma_start(out=out_ext[g * 128:(g + 1) * 128, :],
                                      in_=ot[:])

            for p in range(2):
                flat = []           # (g, d, base, last_of_group)
                base_off = None
                for (gg, pp, chunks, o) in sched:
                    if pp != p:
                        continue
                    if base_off is None:
                        base_off = o
                    for ci, (d, base, take) in enumerate(chunks):
                        flat.append((gg, d, base, ci + 1 == len(chunks)))
                cur_ps = {}
                for w0 in range(0, len(flat), GCHUNK):
                    wchunks = flat[w0:w0 + GCHUNK]
                    ncnk = len(wchunks)
                    gt = gpool.tile([128, GCHUNK * D], BF16, tag="gmsg")
                    n_idx = ncnk * 128
                    q = qctr[0] % NQ
                    qctr[0] += 1
                    o0 = base_off + w0 * 128
                    nc.gpsimd.dma_gather(
                        gt[:, :ncnk * D].rearrange("p (c f) -> p c f", f=D),
                        banks[p],
                        idx_t[:, o0 // 16:(o0 + ncnk * 128) // 16],
                        n_idx, n_idx, D, queue_num=q, single_packet=False,
                    )
                    for ci, (g, d, base, last) in enumerate(wchunks):
                        if g not in cur_ps:
                            segps = ppool.tile([128, D], F32, space="PSUM",
                                               tag="segps",
                                               name=f"segps_{layer}_{p}_{g}")
                            cur_ps[g] = (segps, True)
                        ps, first = cur_ps[g]
                        nc.tensor.matmul(
                            ps[:],
                            lhsT=wext_t[d][:, 127 - base:255 - base],
                            rhs=gt[:, ci * D:(ci + 1) * D],
                            start=first, stop=last,
                        )
                        cur_ps[g] = (ps, False)
                        if last:
                            if p == 0:
                                nc.scalar.activation(
                                    parked[:, g * D:(g + 1) * D], ps[:],
                                    A.Copy)
                            else:
                                epilogue(g, ps)
                            del cur_ps[g]

        # layer 1 (epilogue writes hs2_shard slices)
        message_pass(hs1_ag, h1s_all, layer=1)

        nc.gpsimd.collective_compute(
            "AllGather", mybir.AluOpType.bypass,
            replica_groups=[core_ids],
            ins=[hs2_shard[:]], outs=[hs2_ag[:]],
        )

        # layer 2 (epilogue writes final outputs)
        message_pass(hs2_ag, h2_parked, layer=2)

    nc.compile()
    return nc


# ==================================================================== entry
_CACHE = {}


def kernel(x, edge_index, W1, b1, W2, b2, Wl, bl):
    import ml_dtypes  # noqa: F401  (registers bfloat16 with numpy)

    x = np.asarray(x, dtype=np.float32)
    edge_index = np.asarray(edge_index)
    W1 = np.asarray(W1, dtype=np.float32)
    W2 = np.asarray(W2, dtype=np.float32)
    Wl = np.asarray(Wl, dtype=np.float32)
    b1 = np.asarray(b1, dtype=np.float32)
    b2 = np.asarray(b2, dtype=np.float32)
    bl = np.asarray(bl, dtype=np.float32)

    prep = _host_prep(x, edge_index)
    has_b1 = bool(np.any(b1))
    has_b2 = bool(np.any(b2))

    nc = _build_nc(prep, has_b1, has_b2)

    wl_bcast = np.tile(Wl.reshape(1, D), (128, 1)).astype(np.float32)
    bl_rep = np.full((128, 1), float(bl.reshape(-1)[0]), dtype=np.float32)
    b1_bcast = np.tile(b1.reshape(1, D), (128, 1)).astype(np.float32)
    b2_bcast = np.tile(b2.reshape(1, D), (128, 1)).astype(np.float32)

    import ml_dtypes as mld
    in_maps = []
    for c in range(NC):
        m = {
            "xT": prep["xT_maps"][c],
            "deg_g": prep["deg_maps"][c],
            "idx_all": prep["idx_maps"][c],
            "W1": W1, "W2": W2,
            "Wl_bcast": wl_bcast, "bl_rep": bl_rep,
            "b1_bcast": b1_bcast, "b2_bcast": b2_bcast,
        }
        for d, w in prep["w_ext"].items():
            m[f"w_ext_{d}"] = np.asarray(w, dtype=mld.bfloat16)
        m["ident"] = np.eye(128, dtype=np.float32)
        in_maps.append(m)

    trace = bool(os.environ.get("GNN_TRACE"))
    kw = {}
    if trace:
        kw = dict(trace=True, tmpdir=os.environ.get("GNN_TRACE_DIR") or None)
    res = run_bass_kernel_spmd(nc, in_maps, list(range(NC)), **kw)
    _CACHE["last_result"] = res

    out = np.empty((N_REAL, 1), dtype=np.float32)
    for c in range(NC):
        o = res.results[c]["out"]          # [SHARD, 1], pi order
        pi = prep["pis"][c]
        mask = pi >= 0
        out[c * SHARD_REAL + pi[mask], 0] = o[mask, 0]
    return out


if __name__ == "__main__":
    rng = np.random.default_rng(0)
    x = rng.standard_normal((N_REAL, D), dtype=np.float32)
    ei = rng.integers(0, N_REAL, size=(2, E_EDGES), dtype=np.int64)
    W1 = rng.standard_normal((D, D), dtype=np.float32) / np.sqrt(D)
    W2 = rng.standard_normal((D, D), dtype=np.float32) / np.sqrt(D)
    Wl = rng.standard_normal((D, 1), dtype=np.float32) / np.sqrt(D)
    z = np.zeros(D, dtype=np.float32)
    out = kernel(x=x, edge_index=ei, W1=W1, b1=z, W2=W2, b2=z,
                 Wl=Wl, bl=np.zeros(1, dtype=np.float32))
    print(out.shape, out[:5, 0])
